# revision 3
# baseline (speedup 1.0000x reference)
"""GAT (2-layer, 4-head) on 8 Trainium2 NeuronCores.

Strategy (1D graph/data parallel, per sharding hint):
  - Nodes partitioned into 8 contiguous shards of 6250; each core owns the
    edges whose dst lands in its shard (host sorts edges by dst).
  - Small weight matrices replicated to every core.
  - Per layer, each core computes a node "record" [feat(256) | el(4) | er(4)]
    for its own nodes, then an AllGather replicates the record table so each
    core can gather arbitrary src rows locally.
  - Edge aggregation: edges processed in 128-edge tiles grouped under
    128-dst blocks. Per tile: indirect-DMA gather of src records, edge
    weights w = exp(leaky_relu(el_src + er_dst)), a one-hot dst matrix built
    on the vector engine, and a PE matmul  psum += onehot.T @ (w * feat)
    which performs the segment-sum (softmax numerator and denominator
    accumulate together; the softmax max-subtraction is skipped because the
    logits are tiny and softmax is shift-invariant).
  - Readout is host-fused: sigmoid((h@p1+b1)@p2+b2) == sigmoid(h@(p1@p2)+c).
"""
import math
import numpy as np

import concourse.bass as bass
import concourse.bacc as bacc
import concourse.mybir as mybir
import concourse.tile as tile
from concourse.bass_utils import run_bass_kernel_spmd

# ---------------- problem constants (nn_GAT_36429912605263) ----------------
N = 50000
E = 500000
IN = 256
HID = 64
H = 4
F = 64          # per-head feature dim == HID
NCORES = 8
P = 128
REC = 264       # feat(256) | el(4) | er(4)
f32 = mybir.dt.float32
i32 = mybir.dt.int32


# ---------------------------- device program -------------------------------
def build_program(nloc: int, t_fix: int, n_total: int):
    """Build the SPMD Bass program for one core (same program, per-core data).

    nloc: nodes owned per core; n_total: total nodes (= nloc * NCORES).
    t_fix: edge tiles per 128-dst block (uniform across cores/blocks).
    """
    NB = math.ceil(nloc / P)             # dst blocks per core
    C = NB * t_fix                       # edge-tile columns
    nc = bacc.Bacc(None, target_bir_lowering=False, num_devices=NCORES)

    def din(name, shape, dtype=f32):
        return nc.declare_dram_parameter(name, list(shape), dtype, isOutput=False)

    xTp_d = din("xTp", [P, 2, nloc])            # x shard, transposed+packed
    eW_d = din("eW", [P, 2, HID])               # embed_W packed (256 -> 2x128)
    eb_d = din("eb", [P, HID])                  # embed_b broadcast
    W1_d = din("W1s", [HID, 256])
    W1al_d = din("W1al", [HID, 8])              # W1 @ [al1|ar1] selection
    W2_d = din("W2s", [P, 2, 256])              # W2 packed
    W2al_d = din("W2al", [P, 2, 8])
    b1_d = din("b1b", [P, 256])
    b2_d = din("b2b", [P, 256])
    pW_d = din("pWs", [P, 2, 1])                # p1_W @ p2_W packed
    pb_d = din("pbb", [P, 1])
    iota_d = din("iota2", [P, P])
    ident_d = din("ident", [P, P])
    srcg_d = din("srcg", [P, C], i32)           # global src id per edge slot
    dstl_d = din("dstl", [P, C], i32)           # local dst id (for er gather)
    dstf_d = din("dstf", [P, C])                # dst-in-block id as f32 (-1 pad)
    y_d = nc.declare_dram_parameter("y", [nloc, 1], f32, isOutput=True)

    rec1_loc = nc.dram_tensor("rec1_loc", [nloc, REC], f32)
    rec1_full = nc.dram_tensor("rec1_full", [n_total, REC], f32, addr_space="Shared")
    rec2_loc = nc.dram_tensor("rec2_loc", [nloc, REC], f32)
    rec2_full = nc.dram_tensor("rec2_full", [n_total, REC], f32, addr_space="Shared")
    h1T_loc = nc.dram_tensor("h1T_loc", [P, 2, nloc], f32)

    AF = mybir.ActivationFunctionType
    OP = mybir.AluOpType
    RG = [list(range(NCORES))]

    with tile.TileContext(nc) as tc:
        with (
            tc.tile_pool(name="consts", bufs=1) as cp,
            tc.tile_pool(name="sbuf", bufs=3) as pool,
            tc.tile_pool(name="gpool", bufs=2) as gpool,
            tc.tile_pool(name="psum", bufs=2, space="PSUM") as pp,
            tc.tile_pool(name="psum_u", bufs=2, space="PSUM") as ppu,
        ):
            # ---- load constants once ----
            def const(dram, shape, dtype=f32):
                t = cp.tile(list(shape), dtype, tag=dram.name)
                nc.sync.dma_start(out=t[:], in_=dram[:])
                return t

            eW = const(eW_d, [P, 2, HID])
            eb = const(eb_d, [P, HID])
            W1 = const(W1_d, [HID, 256])
            W1al = const(W1al_d, [HID, 8])
            W2 = const(W2_d, [P, 2, 256])
            W2al = const(W2al_d, [P, 2, 8])
            b1 = const(b1_d, [P, 256])
            b2 = const(b2_d, [P, 256])
            pW = const(pW_d, [P, 2, 1])
            pb = const(pb_d, [P, 1])
            iota2 = const(iota_d, [P, P])
            ident = const(ident_d, [P, P])
            srcg = const(srcg_d, [P, C], i32)
            dstl = const(dstl_d, [P, C], i32)
            dstf = const(dstf_d, [P, C])

            def node_tiles():
                for ntl in range(NB):
                    n0 = ntl * P
                    yield ntl, n0, min(P, nloc - n0)

            # ---------------- phase A: embed + feat1/el1/er1 records -------
            for ntl, n0, pn in node_tiles():
                xt = pool.tile([P, 2, P], f32, tag="xt")
                nc.sync.dma_start(out=xt[:, :, :pn], in_=xTp_d[:, :, n0:n0 + pn])
                ps_h0 = pp.tile([P, HID], f32, tag="ps_small", space="PSUM")
                for k in range(2):
                    nc.tensor.matmul(ps_h0[:pn, :], lhsT=xt[:, k, :pn],
                                     rhs=eW[:, k, :], start=(k == 0), stop=(k == 1))
                h0 = pool.tile([P, HID], f32, tag="h0")
                nc.vector.tensor_tensor(out=h0[:pn, :], in0=ps_h0[:pn, :],
                                        in1=eb[:pn, :], op=OP.add)
                ps_t = pp.tile([P, P], f32, tag="ps_t", space="PSUM")
                nc.tensor.transpose(out=ps_t[:HID, :pn], in_=h0[:pn, :HID],
                                    identity=ident[:pn, :pn])
                h0T = pool.tile([HID, P], f32, tag="h0T")
                nc.scalar.copy(out=h0T[:, :pn], in_=ps_t[:HID, :pn])
                ps_r = pp.tile([P, REC], f32, tag="ps_rec", space="PSUM")
                nc.tensor.matmul(ps_r[:pn, 0:256], lhsT=h0T[:, :pn], rhs=W1[:])
                nc.tensor.matmul(ps_r[:pn, 256:264], lhsT=h0T[:, :pn], rhs=W1al[:])
                rec = pool.tile([P, REC], f32, tag="rec")
                nc.scalar.copy(out=rec[:pn, :], in_=ps_r[:pn, :])
                nc.sync.dma_start(out=rec1_loc[n0:n0 + pn, :], in_=rec[:pn, :])

            # ---- AllGather layer-1 records ----
            nc.gpsimd.collective_compute(
                "AllGather", OP.bypass, replica_groups=RG,
                ins=[rec1_loc[:]], outs=[rec1_full[:]])

            # ---------------- edge aggregation (shared for both layers) ----
            def edge_layer(rec_full, rec_loc, bias_t, is_last):
                for b in range(NB):
                    n0 = b * P
                    pn = min(P, nloc - n0)
                    G = gpool.tile([P, t_fix, REC], f32, tag="G")
                    ER = pool.tile([P, t_fix, H], f32, tag="ER")
                    for t in range(t_fix):
                        col = b * t_fix + t
                        nc.gpsimd.indirect_dma_start(
                            out=G[:, t, :], out_offset=None, in_=rec_full[:],
                            in_offset=bass.IndirectOffsetOnAxis(
                                ap=srcg[:, col:col + 1], axis=0))
                        nc.gpsimd.indirect_dma_start(
                            out=ER[:, t, :], out_offset=None, in_=rec_loc[:],
                            in_offset=bass.IndirectOffsetOnAxis(
                                ap=dstl[:, col:col + 1], axis=0),
                            element_offset=260)
                    # w = exp(leaky_relu(el + er))  (batched over the block)
                    wb = pool.tile([P, t_fix, H], f32, tag="wb")
                    nc.vector.tensor_tensor(out=wb[:], in0=G[:, :, 256:260],
                                            in1=ER[:], op=OP.add)
                    wt = pool.tile([P, t_fix, H], f32, tag="wt")
                    nc.vector.tensor_scalar_mul(wt[:], wb[:], 0.2)
                    nc.vector.tensor_tensor(out=wb[:], in0=wb[:], in1=wt[:],
                                            op=OP.max)
                    nc.scalar.activation(wb[:], wb[:], AF.Exp)
                    # one-hot dst matrices, 4 tiles per DVE op
                    OH = gpool.tile([P, t_fix, P], f32, tag="OH")
                    for t0 in range(0, t_fix, 4):
                        tw = min(4, t_fix - t0)
                        c0 = b * t_fix + t0
                        nc.vector.tensor_tensor(
                            out=OH[:, t0:t0 + tw, :],
                            in0=dstf[:, c0:c0 + tw, None].to_broadcast([P, tw, P]),
                            in1=iota2[:, None, :].to_broadcast([P, tw, P]),
                            op=OP.is_equal)
                    psU = ppu.tile([P, REC], f32, tag="psU", space="PSUM")
                    for t in range(t_fix):
                        M = pool.tile([P, REC], f32, tag="M")
                        nc.vector.tensor_tensor(
                            out=M[:, 0:256].rearrange("p (h f) -> p h f", h=H),
                            in0=G[:, t, 0:256].rearrange("p (h f) -> p h f", h=H),
                            in1=wb[:, t, :, None].to_broadcast([P, H, F]),
                            op=OP.mult)
                        nc.vector.tensor_copy(
                            out=M[:, 256:264].rearrange("p (x h) -> p x h", x=2),
                            in_=wb[:, t, None, :].to_broadcast([P, 2, H]))
                        nc.tensor.matmul(psU[:], lhsT=OH[:, t, :], rhs=M[:],
                                         start=(t == 0), stop=(t == t_fix - 1))
                    # h = relu(U / s + bias)
                    r = pool.tile([P, H], f32, tag="r")
                    nc.vector.tensor_scalar_max(r[:], psU[:, 260:264], 1e-30)
                    nc.vector.reciprocal(r[:], r[:])
                    h = pool.tile([P, 256], f32, tag="h")
                    nc.vector.tensor_tensor(
                        out=h[:, :].rearrange("p (h f) -> p h f", h=H),
                        in0=psU[:, 0:256].rearrange("p (h f) -> p h f", h=H),
                        in1=r[:, :, None].to_broadcast([P, H, F]), op=OP.mult)
                    nc.vector.tensor_tensor(out=h[:], in0=h[:], in1=bias_t[:],
                                            op=OP.add)
                    nc.vector.tensor_scalar_max(h[:], h[:], 0.0)
                    # transpose h for the next contraction
                    hT = pool.tile([P, 2, P], f32, tag="hT")
                    for k in range(2):
                        ps_t2 = pp.tile([P, P], f32, tag="ps_t", space="PSUM")
                        nc.tensor.transpose(out=ps_t2[:, :pn],
                                            in_=h[:pn, k * P:(k + 1) * P],
                                            identity=ident[:pn, :pn])
                        nc.scalar.copy(out=hT[:, k, :pn], in_=ps_t2[:, :pn])
                    if not is_last:
                        nc.sync.dma_start(out=h1T_loc[:, :, n0:n0 + pn],
                                          in_=hT[:, :, :pn])
                    else:
                        ps_y = pp.tile([P, 1], f32, tag="ps_small", space="PSUM")
                        for k in range(2):
                            nc.tensor.matmul(ps_y[:pn, :], lhsT=hT[:, k, :pn],
                                             rhs=pW[:, k, :],
                                             start=(k == 0), stop=(k == 1))
                        sig = pool.tile([P, 1], f32, tag="sig")
                        nc.scalar.activation(sig[:pn, :], ps_y[:pn, :],
                                             AF.Sigmoid, bias=pb[:pn, :])
                        nc.sync.dma_start(out=y_d[n0:n0 + pn, :], in_=sig[:pn, :])

            edge_layer(rec1_full, rec1_loc, b1, is_last=False)

            # ---------------- phase C: feat2/el2/er2 records ----------------
            for ntl, n0, pn in node_tiles():
                h1t = pool.tile([P, 2, P], f32, tag="h1t")
                nc.sync.dma_start(out=h1t[:, :, :pn], in_=h1T_loc[:, :, n0:n0 + pn])
                ps_r = pp.tile([P, REC], f32, tag="ps_rec", space="PSUM")
                for k in range(2):
                    nc.tensor.matmul(ps_r[:pn, 0:256], lhsT=h1t[:, k, :pn],
                                     rhs=W2[:, k, :], start=(k == 0), stop=(k == 1))
                for k in range(2):
                    nc.tensor.matmul(ps_r[:pn, 256:264], lhsT=h1t[:, k, :pn],
                                     rhs=W2al[:, k, :], start=(k == 0), stop=(k == 1))
                rec = pool.tile([P, REC], f32, tag="rec")
                nc.scalar.copy(out=rec[:pn, :], in_=ps_r[:pn, :])
                nc.sync.dma_start(out=rec2_loc[n0:n0 + pn, :], in_=rec[:pn, :])

            nc.gpsimd.collective_compute(
                "AllGather", OP.bypass, replica_groups=RG,
                ins=[rec2_loc[:]], outs=[rec2_full[:]])

            edge_layer(rec2_full, rec2_loc, b2, is_last=True)

    nc.finalize()
    return nc


# --------------------------- host-side helpers -----------------------------
def _prep_edges(src, dst, nloc, n_cores):
    """Sort/pad edges per core into uniform [P, NB*t_fix] slot arrays."""
    NB = math.ceil(nloc / P)
    per_core = []
    t_fix = 1
    for r in range(n_cores):
        lo, hi = r * nloc, (r + 1) * nloc
        m = (dst >= lo) & (dst < hi)
        s_r, d_r = src[m], dst[m] - lo
        order = np.argsort(d_r, kind="stable")
        s_r, d_r = s_r[order], d_r[order]
        blk = d_r // P
        cnt = np.bincount(blk, minlength=NB)
        t_fix = max(t_fix, int(np.ceil(cnt.max() / P)))
        per_core.append((s_r, d_r, blk, cnt))
    C = NB * t_fix
    srcg = np.zeros((n_cores, P, C), np.int32)
    dstl = np.zeros((n_cores, P, C), np.int32)
    dstf = np.full((n_cores, P, C), -1.0, np.float32)
    for r, (s_r, d_r, blk, cnt) in enumerate(per_core):
        starts = np.zeros(NB + 1, np.int64)
        np.cumsum(cnt, out=starts[1:])
        for b in range(NB):
            e0, e1 = starts[b], starts[b + 1]
            n_e = e1 - e0
            sl_src = np.zeros(t_fix * P, np.int32)
            sl_dst = np.zeros(t_fix * P, np.int32)
            sl_flt = np.full(t_fix * P, -1.0, np.float32)
            sl_src[:n_e] = s_r[e0:e1]
            sl_dst[:n_e] = d_r[e0:e1]
            sl_flt[:n_e] = (d_r[e0:e1] - b * P).astype(np.float32)
            c0 = b * t_fix
            srcg[r, :, c0:c0 + t_fix] = sl_src.reshape(t_fix, P).T
            dstl[r, :, c0:c0 + t_fix] = sl_dst.reshape(t_fix, P).T
            dstf[r, :, c0:c0 + t_fix] = sl_flt.reshape(t_fix, P).T
    return t_fix, srcg, dstl, dstf


def _pack_rows(w):
    """[256, X] -> [128, 2, X] with [p, k, :] = w[128k+p, :]."""
    return np.ascontiguousarray(w.reshape(2, P, -1).transpose(1, 0, 2))


_CACHE = {}


def kernel(x, src, dst, embed_W, embed_b, W1, al1, ar1, b1,
           W2, al2, ar2, b2, p1_W, p1_b, p2_W, p2_b):
    x = np.asarray(x); src = np.asarray(src, np.int32); dst = np.asarray(dst, np.int32)
    n_total = x.shape[0]
    nloc = n_total // NCORES
    t_fix, srcg, dstl, dstf = _prep_edges(src, dst, nloc, NCORES)

    key = (n_total, nloc, t_fix)
    if key not in _CACHE:
        _CACHE[key] = build_program(nloc, t_fix, n_total)
    nc = _CACHE[key]

    # host-derived weights
    def sel(al, ar):
        s = np.zeros((H * F, 2 * H), np.float32)
        for h in range(H):
            s[h * F:(h + 1) * F, h] = al[h]
            s[h * F:(h + 1) * F, H + h] = ar[h]
        return s

    W1al = np.asarray(W1) @ sel(np.asarray(al1), np.asarray(ar1))      # [64, 8]
    W2al = np.asarray(W2) @ sel(np.asarray(al2), np.asarray(ar2))      # [256, 8]
    pW = np.asarray(p1_W) @ np.asarray(p2_W)                            # [256, 1]
    pb = float((np.asarray(p1_b) @ np.asarray(p2_W) + np.asarray(p2_b)).reshape(-1)[0])

    bcast = lambda v, n: np.ascontiguousarray(
        np.broadcast_to(np.asarray(v, np.float32).reshape(1, n), (P, n)))
    common = {
        "eW": _pack_rows(np.asarray(embed_W, np.float32)),
        "eb": bcast(embed_b, HID),
        "W1s": np.ascontiguousarray(np.asarray(W1, np.float32)),
        "W1al": np.ascontiguousarray(W1al),
        "W2s": _pack_rows(np.asarray(W2, np.float32)),
        "W2al": _pack_rows(W2al),
        "b1b": bcast(b1, 256),
        "b2b": bcast(b2, 256),
        "pWs": _pack_rows(pW),
        "pbb": np.full((P, 1), pb, np.float32),
        "iota2": np.ascontiguousarray(
            np.broadcast_to(np.arange(P, dtype=np.float32)[None, :], (P, P))),
        "ident": np.eye(P, dtype=np.float32),
    }
    in_maps = []
    for r in range(NCORES):
        xs = np.asarray(x[r * nloc:(r + 1) * nloc], np.float32)  # [nloc, 256]
        xTp = np.ascontiguousarray(xs.T.reshape(2, P, nloc).transpose(1, 0, 2))
        in_maps.append({**common, "xTp": xTp, "srcg": srcg[r],
                        "dstl": dstl[r], "dstf": dstf[r]})

    res = run_bass_kernel_spmd(nc, in_maps, core_ids=list(range(NCORES)))
    y = np.concatenate([res.results[r]["y"] for r in range(NCORES)], axis=0)
    return y.astype(np.float32)
